# revision 18
# baseline (speedup 1.0000x reference)
"""MultiHeadAttentionPool3D on 8 Trainium2 NeuronCores.

Math (per batch b):
  scores[hq, s] = scale * (q_eff[hq, :] @ x[b, :, s])     (key-projection folded into
                                                           the queries; per-row bias
                                                           terms cancel in softmax)
  p = exp(scores)       (no max-subtraction: scores ~ N(0,1), fp32-safe)
  l[hq] = sum_s p[hq, s];   y[hq, c] = sum_s p[hq, s] * x[b, c, s]
  pooled = y / l  -> tiny epilogue (value proj, Wo, layernorm) on host.

Sharding: core = b * 2 + s_half  (4 batches x 2 halves of S=32768).

v2 design (instruction-count minimized; ~250 instrs/core):
  - host passes TWO fp16 layouts of the shard: x16 [C, S_loc] for the score
    matmul, and a pre-tiled transposed plane xt [n_chunks, 128, sbpc*257]
    (tile j = x[:, j*128:(j+1)*128].T with an appended ones column, which makes
    the softmax denominator fall out of the same matmul that computes y).
  - scores: 64 matmuls (stationary q_effT c-half, moving x16 [128, 512]).
  - p = Exp(scale * scores): 32 ScalarE activations, fp32 PSUM -> fp16 SBUF.
  - pT: ONE xbar transpose-DMA per chunk ([32, 4096] -> [128, 32, 32]).
  - y_aug[32, 257] += pT_j.T @ xt_j over all 128 s-blocks (PSUM-accumulated);
    column 256 is l. Chunk-level software pipelining (mm2 lags one chunk).
"""

import sys

if "/opt/trn_rl_repo" not in sys.path:
    sys.path.insert(0, "/opt/trn_rl_repo")

import numpy as np

NUM_HEADS = 8
OUT_FEATURES = 512
NUM_QUERIES = 4
C = 256
HEAD_DIM = OUT_FEATURES // NUM_HEADS
LN_EPS = 1e-5
B = 4
S = 32 * 32 * 32
N_CORES = 8
S_LOC = S // 2  # shard: (batch, half of spatial axis)
HQ = NUM_HEADS * NUM_QUERIES  # 32 fused query rows, hq = h*NUM_QUERIES + q
SCALE = HEAD_DIM ** -0.5
CHUNK = 4096

_NC_CACHE = {}


def _build_nc(s_loc=S_LOC, chunk=CHUNK, loop_n=1, x_f32=False, multi_queue=True):
    import concourse.bass as bass
    import concourse.tile as tile
    from concourse import bacc, mybir
    import contextlib

    f32 = mybir.dt.float32
    f16 = mybir.dt.float16
    xdt = f32 if x_f32 else f16
    Exp = mybir.ActivationFunctionType.Exp

    assert s_loc % chunk == 0 and chunk % 512 == 0
    n_ch = s_loc // chunk
    sbpc = chunk // 128     # s-blocks (128-wide) per chunk
    nsc = chunk // 512      # 512-wide score tiles per chunk
    n_sb = s_loc // 128
    W = 256                 # xt tile width (channels); l comes from accum_out

    nc = bacc.Bacc("TRN2", target_bir_lowering=False, debug=False,
                   num_devices=N_CORES)
    x_d = nc.dram_tensor("x", [C, s_loc], xdt, kind="ExternalInput")
    xt_d = nc.dram_tensor("xt", [n_ch, 128, sbpc * W], f16, kind="ExternalInput")
    qT_d = nc.dram_tensor("qT", [C, HQ], xdt, kind="ExternalInput")
    y_d = nc.dram_tensor("y", [HQ, W + 1], f32, kind="ExternalOutput")

    with tile.TileContext(nc) as tc:
        with (
            tc.tile_pool(name="const", bufs=1) as constp,
            tc.tile_pool(name="xstage", bufs=4) as xstage,
            tc.tile_pool(name="xtstage", bufs=3) as xtstage,
            tc.tile_pool(name="pstage", bufs=n_ch) as pstage,
            tc.tile_pool(name="ptstage", bufs=n_ch) as ptstage,
            tc.tile_pool(name="lbuf", bufs=1) as lbufp,
            tc.tile_pool(name="outp", bufs=2) as outp,
            tc.tile_pool(name="ps_sc", bufs=4, space="PSUM") as ps_sc,
            tc.tile_pool(name="ps_y", bufs=1, space="PSUM") as ps_yp,
        ):
            qt0 = constp.tile([128, HQ], xdt)
            nc.sync.dma_start(qt0[:], qT_d[0:128, :])
            qt1 = constp.tile([128, HQ], xdt)
            nc.sync.dma_start(qt1[:], qT_d[128:256, :])

            def iter_scope():
                if loop_n > 1:
                    E = mybir.EngineType
                    return tc.For_i(0, loop_n, 1,
                                    hint_engines=(E.PE, E.DVE, E.Activation,
                                                  E.SP, E.Pool))
                return contextlib.nullcontext()

            q0 = nc.sync
            q1 = nc.scalar if multi_queue else nc.sync
            with iter_scope():
                psum_y = ps_yp.tile([HQ, W], f32, tag="psy")
                lbuf = lbufp.tile([HQ, n_ch * nsc], f32, tag="lb")

                tiles = {}  # ch -> (pt_c, xt_c)
                trq = {}    # ch -> (p_c, pt_c) pending transpose

                def emit_mm2(ch):
                    pt_c, xt_c = tiles[ch]
                    for j in range(sbpc):
                        sb = ch * sbpc + j
                        nc.tensor.matmul(psum_y[:],
                                         pt_c[:, j * HQ:(j + 1) * HQ],
                                         xt_c[:, j * W:(j + 1) * W],
                                         start=(sb == 0), stop=(sb == n_sb - 1))

                def emit_tr(ch, eng):
                    p_c, pt_c = trq.pop(ch)
                    eng.dma_start_transpose(
                        pt_c.rearrange("p (j q) -> p j q", j=sbpc), p_c[:])

                for ch in range(n_ch):
                    o = ch * chunk
                    qa, qb = (q0, q1) if ch % 2 == 0 else (q1, q0)
                    # previous chunk's p-transpose leads this window on qa
                    if ch >= 1:
                        emit_tr(ch - 1, qa)
                    # this chunk's x pair on qa, xt on qb
                    xc0 = xstage.tile([128, chunk], xdt, tag="xc0")
                    qa.dma_start(xc0[:], x_d[0:128, o:o + chunk])
                    xc1 = xstage.tile([128, chunk], xdt, tag="xc1")
                    qa.dma_start(xc1[:], x_d[128:256, o:o + chunk])
                    xt_c = xtstage.tile([128, sbpc * W], f16, tag="xt")
                    if ch < n_ch - 1:
                        qb.dma_start(xt_c[:], xt_d[ch])
                    else:
                        # split the last xt so the final mm2s chase the DMA
                        quarter = sbpc * W // 4
                        for s4 in range(4):
                            qb.dma_start(xt_c[:, s4 * quarter:(s4 + 1) * quarter],
                                         xt_d[ch][:, s4 * quarter:(s4 + 1) * quarter])

                    p_c = pstage.tile([HQ, chunk], f16, tag="pc")
                    for t in range(nsc):
                        so = t * 512
                        ps = ps_sc.tile([HQ, 512], f32, tag="ps")
                        nc.tensor.matmul(ps[:], qt0[:], xc0[:, so:so + 512],
                                         start=True, stop=False)
                        nc.tensor.matmul(ps[:], qt1[:], xc1[:, so:so + 512],
                                         start=False, stop=True)
                        li = ch * nsc + t
                        nc.scalar.activation(p_c[:, so:so + 512], ps[:], Exp,
                                             scale=SCALE,
                                             accum_out=lbuf[:, li:li + 1])
                    pt_c = ptstage.tile([128, sbpc * HQ], f16, tag="pt")
                    trq[ch] = (p_c, pt_c)
                    tiles[ch] = (pt_c, xt_c)
                    if ch >= 1:
                        emit_mm2(ch - 1)
                emit_tr(n_ch - 1, q0)
                emit_mm2(n_ch - 1)

                y_t = outp.tile([HQ, W + 1], f32, tag="yt")
                nc.vector.tensor_copy(y_t[:, 0:W], psum_y[:])
                nc.vector.reduce_sum(y_t[:, W:W + 1], lbuf[:],
                                     axis=mybir.AxisListType.X)
                nc.sync.dma_start(y_d[:], y_t[:])

    nc.compile()
    return nc


def _get_nc(loop_n=1, x_f32=False, chunk=CHUNK, multi_queue=True):
    key = (S_LOC, loop_n, x_f32, chunk, multi_queue)
    if key not in _NC_CACHE:
        _NC_CACHE[key] = _build_nc(loop_n=loop_n, x_f32=x_f32, chunk=chunk,
                                   multi_queue=multi_queue)
    return _NC_CACHE[key]


def _shard_inputs(shard, qT, s_loc=S_LOC, chunk=CHUNK, x_f32=False):
    """shard: [C, s_loc] fp32 -> in_map for one core."""
    n_ch = s_loc // chunk
    sbpc = chunk // 128
    x16 = shard.astype(np.float32 if x_f32 else np.float16)
    # tile j of chunk ch = shard[:, ch*chunk + j*128 : +128].T
    xt = np.ascontiguousarray(
        shard.T.reshape(n_ch, sbpc, 128, C).astype(np.float16)
        .transpose(0, 2, 1, 3).reshape(n_ch, 128, sbpc * C))
    return {"x": np.ascontiguousarray(x16), "xt": xt,
            "qT": qT.astype(np.float32 if x_f32 else np.float16)}


def _prepare_in_maps(x, queries, Wk, x_f32=False):
    xf = np.ascontiguousarray(np.asarray(x, np.float32).reshape(B, C, S))
    qr = np.asarray(queries, np.float32).reshape(NUM_QUERIES, NUM_HEADS, HEAD_DIM)
    Wkr = np.asarray(Wk, np.float32).reshape(NUM_HEADS, HEAD_DIM, C)
    # q_eff[h*NQ+q, c] = sum_d q[q,h,d] * Wk[h*hd+d, c]
    q_eff = np.einsum("qhd,hdc->hqc", qr, Wkr).reshape(HQ, C)
    qT = np.ascontiguousarray(q_eff.T.astype(np.float32))
    in_maps = []
    for core in range(N_CORES):
        b, half = divmod(core, 2)
        shard = np.ascontiguousarray(xf[b, :, half * S_LOC:(half + 1) * S_LOC])
        in_maps.append(_shard_inputs(shard, qT, x_f32=x_f32))
    return in_maps


def _epilogue(Y, L, Wv, bv, Wo, bo, gamma, beta):
    """Y [B, HQ, C], L [B, HQ] -> final [B, OUT_FEATURES]."""
    pooled = (Y / L[:, :, None]).reshape(B, NUM_HEADS, NUM_QUERIES, C)
    Wvr = np.asarray(Wv, np.float32).reshape(NUM_HEADS, HEAD_DIM, C)
    att = np.einsum("hdc,bhqc->bhqd", Wvr, pooled)
    att += np.asarray(bv, np.float32).reshape(1, NUM_HEADS, 1, HEAD_DIM)
    multi = att.transpose(0, 2, 1, 3).reshape(B, NUM_QUERIES * OUT_FEATURES)
    out = multi @ np.asarray(Wo, np.float32).T + np.asarray(bo, np.float32)
    mu = out.mean(-1, keepdims=True)
    var = ((out - mu) ** 2).mean(-1, keepdims=True)
    out = (out - mu) / np.sqrt(var + LN_EPS)
    out = out * np.asarray(gamma, np.float32) + np.asarray(beta, np.float32)
    return out.astype(np.float32)


def kernel(x, queries, Wk, bk, Wv, bv, Wo, bo, gamma, beta):
    from concourse.bass_utils import run_bass_kernel_spmd

    in_maps = _prepare_in_maps(x, queries, Wk)
    nc = _get_nc()
    res = run_bass_kernel_spmd(nc, in_maps, list(range(N_CORES))).results
    Y = np.zeros((B, HQ, C), np.float32)
    L = np.zeros((B, HQ), np.float32)
    for core in range(N_CORES):
        b = core // 2
        Y[b] += res[core]["y"][:, :256]
        L[b] += res[core]["y"][:, 256]
    return _epilogue(Y, L, Wv, bv, Wo, bo, gamma, beta)


# revision 30
# speedup vs baseline: 1.0415x; 1.0415x over previous
"""MultiHeadAttentionPool3D on 8 Trainium2 NeuronCores.

Math (per batch b):
  scores[hq, s] = scale * (q_eff[hq, :] @ x[b, :, s])     (key-projection folded into
                                                           the queries; per-row bias
                                                           terms cancel in softmax)
  p = exp(scores)       (no max-subtraction: scores ~ N(0,1), fp32-safe)
  l[hq] = sum_s p[hq, s];   y[hq, c] = sum_s p[hq, s] * x[b, c, s]
  pooled = y / l  -> tiny epilogue (value proj, Wo, layernorm) on host.

Sharding: core = b * 2 + s_half  (4 batches x 2 halves of S=32768).

v2 design (instruction-count minimized; ~250 instrs/core):
  - host passes TWO fp16 layouts of the shard: x16 [C, S_loc] for the score
    matmul, and a pre-tiled transposed plane xt [n_chunks, 128, sbpc*257]
    (tile j = x[:, j*128:(j+1)*128].T with an appended ones column, which makes
    the softmax denominator fall out of the same matmul that computes y).
  - scores: 64 matmuls (stationary q_effT c-half, moving x16 [128, 512]).
  - p = Exp(scale * scores): 32 ScalarE activations, fp32 PSUM -> fp16 SBUF.
  - pT: ONE xbar transpose-DMA per chunk ([32, 4096] -> [128, 32, 32]).
  - y_aug[32, 257] += pT_j.T @ xt_j over all 128 s-blocks (PSUM-accumulated);
    column 256 is l. Chunk-level software pipelining (mm2 lags one chunk).
"""

import sys

if "/opt/trn_rl_repo" not in sys.path:
    sys.path.insert(0, "/opt/trn_rl_repo")

import numpy as np

NUM_HEADS = 8
OUT_FEATURES = 512
NUM_QUERIES = 4
C = 256
HEAD_DIM = OUT_FEATURES // NUM_HEADS
LN_EPS = 1e-5
B = 4
S = 32 * 32 * 32
N_CORES = 8
S_LOC = S // 2  # shard: (batch, half of spatial axis)
HQ = NUM_HEADS * NUM_QUERIES  # 32 fused query rows, hq = h*NUM_QUERIES + q
SCALE = HEAD_DIM ** -0.5
CHUNK = 2048

_NC_CACHE = {}


def _build_nc(s_loc=S_LOC, chunk=2048, loop_n=1, x_f32=False, multi_queue=True):
    import concourse.bass as bass
    import concourse.tile as tile
    from concourse import bacc, mybir
    import contextlib

    f32 = mybir.dt.float32
    f16 = mybir.dt.float16
    xdt = f32 if x_f32 else f16
    Exp = mybir.ActivationFunctionType.Exp

    assert s_loc % chunk == 0 and chunk % 512 == 0
    n_ch = s_loc // chunk
    sbpc = chunk // 128     # s-blocks (128-wide) per chunk
    nsc = chunk // 512      # 512-wide score tiles per chunk
    n_sb = s_loc // 128
    W = 256                 # xt tile width (channels); l comes from accum_out

    nc = bacc.Bacc("TRN2", target_bir_lowering=False, debug=False,
                   num_devices=N_CORES)
    x_d = nc.dram_tensor("x", [C, s_loc], xdt, kind="ExternalInput")
    # flat transposed plane: column block j (width W) = x[:, j*128:(j+1)*128].T
    xt_d = nc.dram_tensor("xt", [128, n_sb * W], f16, kind="ExternalInput")
    qT_d = nc.dram_tensor("qT", [C, HQ], xdt, kind="ExternalInput")
    # rows 32g+hq: partial y (sub-accumulator g); col 256: partial l
    y_d = nc.dram_tensor("y", [128, W + 1], f32, kind="ExternalOutput")

    with tile.TileContext(nc) as tc:
        with (
            tc.tile_pool(name="const", bufs=1) as constp,
            tc.tile_pool(name="xstage", bufs=4) as xstage,
            tc.tile_pool(name="xtstage", bufs=3) as xtstage,
            tc.tile_pool(name="pstage", bufs=n_ch) as pstage,
            tc.tile_pool(name="ptstage", bufs=n_ch) as ptstage,
            tc.tile_pool(name="lbuf", bufs=1) as lbufp,
            tc.tile_pool(name="outp", bufs=2) as outp,
            tc.tile_pool(name="ps_sc", bufs=4, space="PSUM") as ps_sc,
            tc.tile_pool(name="ps_y", bufs=1, space="PSUM") as ps_yp,
        ):
            qt0 = constp.tile([128, HQ], xdt)
            nc.sync.dma_start(qt0[:], qT_d[0:128, :])
            qt1 = constp.tile([128, HQ], xdt)
            nc.sync.dma_start(qt1[:], qT_d[128:256, :])

            def iter_scope():
                if loop_n > 1:
                    E = mybir.EngineType
                    return tc.For_i(0, loop_n, 1,
                                    hint_engines=(E.PE, E.DVE, E.Activation,
                                                  E.SP, E.Pool))
                return contextlib.nullcontext()

            q0 = nc.sync
            q1 = nc.scalar if multi_queue else nc.sync
            with iter_scope():
                # Column-packed (tile_position) pipeline over 2048-s segments.
                # Each segment: 4 score-tiles stacked on PSUM partition groups,
                # one [128,512] exp, one xbar transpose, 16 col-packed mm2s
                # into 4 psum_y sub-accumulators (summed on the host).
                seg = chunk          # s width per segment window
                nt = seg // 512      # score tiles per segment (= col groups used)
                spseg = seg // 128   # s-blocks per segment
                nseg = s_loc // seg
                LAG = 1
                assert nt == 4, "segment must be 2048 wide (4 col groups)"

                psum_y = ps_yp.tile([128, W], f32, tag="psy")
                lbuf = lbufp.tile([128, nseg], f32, tag="lb")

                pts = {}
                xts = {}

                def load_x(k):
                    o = k * seg
                    xc0 = xstage.tile([128, seg], xdt, tag="xc0")
                    q0.dma_start(xc0[:], x_d[0:128, o:o + seg])
                    xc1 = xstage.tile([128, seg], xdt, tag="xc1")
                    q0.dma_start(xc1[:], x_d[128:256, o:o + seg])
                    return xc0, xc1

                def emit_compute(k, xc0, xc1):
                    ps4 = ps_sc.tile([128, 512], f32, tag="ps")
                    for g in range(4):
                        nc.tensor.matmul(ps4[32 * g:32 * (g + 1), :], qt0[:],
                                         xc0[:, g * 512:(g + 1) * 512],
                                         start=True, stop=False,
                                         tile_position=(0, 32 * g),
                                         skip_group_check=True)
                    for g in range(4):
                        nc.tensor.matmul(ps4[32 * g:32 * (g + 1), :], qt1[:],
                                         xc1[:, g * 512:(g + 1) * 512],
                                         start=False, stop=True,
                                         tile_position=(0, 32 * g),
                                         skip_group_check=True)
                    p_c = pstage.tile([128, 512], f16, tag="pc")
                    nc.scalar.activation(p_c[:], ps4[:], Exp, scale=SCALE,
                                         accum_out=lbuf[:, k:k + 1])
                    pt_c = ptstage.tile([128, 512], f16, tag="pt")
                    q1.dma_start_transpose(
                        pt_c.rearrange("p (j c) -> p j c", j=4), p_c[:])
                    pts[k] = pt_c

                def load_xt(k, parts=1):
                    o2 = k * spseg * W
                    xt_c = xtstage.tile([128, spseg * W], f16, tag="xt")
                    segw = spseg * W // parts
                    for i in range(parts):
                        q1.dma_start(xt_c[:, i * segw:(i + 1) * segw],
                                     xt_d[:, o2 + i * segw:o2 + (i + 1) * segw])
                    xts[k] = xt_c

                def emit_mm2(k):
                    # pT layout: [128, (j: s-block within 512) x (32t+hq)]
                    pt_c, xt_c = pts.pop(k), xts.pop(k)
                    for j16 in range(spseg):
                        t, j = divmod(j16, 4)
                        sb = k * spseg + j16
                        g = sb % 4
                        nc.tensor.matmul(
                            psum_y[32 * g:32 * (g + 1), :],
                            pt_c[:, j * 128 + 32 * t:j * 128 + 32 * t + 32],
                            xt_c[:, j16 * W:(j16 + 1) * W],
                            start=(sb < 4), stop=(sb >= n_sb - 4),
                            tile_position=(0, 32 * g),
                            skip_group_check=True)

                for k in range(nseg):
                    xc0, xc1 = load_x(k)
                    emit_compute(k, xc0, xc1)
                    if k >= LAG:
                        load_xt(k - LAG)
                        emit_mm2(k - LAG)
                for k in range(nseg - LAG, nseg):
                    load_xt(k, parts=2)
                    emit_mm2(k)

                y_t = outp.tile([128, W + 1], f32, tag="yt")
                nc.vector.tensor_copy(y_t[:, 0:W], psum_y[:])
                nc.vector.reduce_sum(y_t[:, W:W + 1], lbuf[:],
                                     axis=mybir.AxisListType.X)
                nc.sync.dma_start(y_d[:], y_t[:])

    nc.compile()
    return nc


def _get_nc(loop_n=1, x_f32=False, chunk=CHUNK, multi_queue=True):
    key = (S_LOC, loop_n, x_f32, chunk, multi_queue)
    if key not in _NC_CACHE:
        _NC_CACHE[key] = _build_nc(loop_n=loop_n, x_f32=x_f32, chunk=chunk,
                                   multi_queue=multi_queue)
    return _NC_CACHE[key]


def _shard_inputs(shard, qT, s_loc=S_LOC, chunk=CHUNK, x_f32=False):
    """shard: [C, s_loc] fp32 -> in_map for one core."""
    n_sb = s_loc // 128
    x16 = shard.astype(np.float32 if x_f32 else np.float16)
    # flat transposed plane: column block j (width C) = shard[:, j*128:+128].T
    xt = np.ascontiguousarray(
        shard.T.reshape(n_sb, 128, C).astype(np.float16)
        .transpose(1, 0, 2).reshape(128, n_sb * C))
    return {"x": np.ascontiguousarray(x16), "xt": xt,
            "qT": qT.astype(np.float32 if x_f32 else np.float16)}


def _prepare_in_maps(x, queries, Wk, x_f32=False):
    xf = np.ascontiguousarray(np.asarray(x, np.float32).reshape(B, C, S))
    qr = np.asarray(queries, np.float32).reshape(NUM_QUERIES, NUM_HEADS, HEAD_DIM)
    Wkr = np.asarray(Wk, np.float32).reshape(NUM_HEADS, HEAD_DIM, C)
    # q_eff[h*NQ+q, c] = sum_d q[q,h,d] * Wk[h*hd+d, c]
    q_eff = np.einsum("qhd,hdc->hqc", qr, Wkr).reshape(HQ, C)
    qT = np.ascontiguousarray(q_eff.T.astype(np.float32))
    in_maps = []
    for core in range(N_CORES):
        b, half = divmod(core, 2)
        shard = np.ascontiguousarray(xf[b, :, half * S_LOC:(half + 1) * S_LOC])
        in_maps.append(_shard_inputs(shard, qT, x_f32=x_f32))
    return in_maps


def _epilogue(Y, L, Wv, bv, Wo, bo, gamma, beta):
    """Y [B, HQ, C], L [B, HQ] -> final [B, OUT_FEATURES]."""
    pooled = (Y / L[:, :, None]).reshape(B, NUM_HEADS, NUM_QUERIES, C)
    Wvr = np.asarray(Wv, np.float32).reshape(NUM_HEADS, HEAD_DIM, C)
    att = np.einsum("hdc,bhqc->bhqd", Wvr, pooled)
    att += np.asarray(bv, np.float32).reshape(1, NUM_HEADS, 1, HEAD_DIM)
    multi = att.transpose(0, 2, 1, 3).reshape(B, NUM_QUERIES * OUT_FEATURES)
    out = multi @ np.asarray(Wo, np.float32).T + np.asarray(bo, np.float32)
    mu = out.mean(-1, keepdims=True)
    var = ((out - mu) ** 2).mean(-1, keepdims=True)
    out = (out - mu) / np.sqrt(var + LN_EPS)
    out = out * np.asarray(gamma, np.float32) + np.asarray(beta, np.float32)
    return out.astype(np.float32)


def kernel(x, queries, Wk, bk, Wv, bv, Wo, bo, gamma, beta):
    from concourse.bass_utils import run_bass_kernel_spmd

    in_maps = _prepare_in_maps(x, queries, Wk)
    nc = _get_nc()
    res = run_bass_kernel_spmd(nc, in_maps, list(range(N_CORES))).results
    Y = np.zeros((B, HQ, C), np.float32)
    L = np.zeros((B, HQ), np.float32)
    for core in range(N_CORES):
        b = core // 2
        yg = res[core]["y"].reshape(4, HQ, C + 1).sum(axis=0)
        Y[b] += yg[:, :C]
        L[b] += yg[:, C]
    return _epilogue(Y, L, Wv, bv, Wo, bo, gamma, beta)


# revision 32
# speedup vs baseline: 1.5898x; 1.5265x over previous
"""MultiHeadAttentionPool3D on 8 Trainium2 NeuronCores.

Math (per batch b):
  scores[hq, s] = scale * (q_eff[hq, :] @ x[b, :, s])     (key-projection folded into
                                                           the queries; per-row bias
                                                           terms cancel in softmax)
  p = exp(scores)       (no max-subtraction: scores ~ N(0,1), fp32-safe)
  l[hq] = sum_s p[hq, s];   y[hq, c] = sum_s p[hq, s] * x[b, c, s]
  pooled = y / l  -> tiny epilogue (value proj, Wo, layernorm) on host.

Sharding: core = b * 2 + s_half  (4 batches x 2 halves of S=32768).

v9 design (both matmuls in small-output orientation):
  - host passes TWO fp16 layouts of the shard: x [C, S_loc] (c on partitions)
    and a flat transposed plane xt [128, n_sb*256] (s on partitions; column
    block j = x[:, j*128:(j+1)*128].T).
  - scoresT per s-block DIRECTLY via x-tile-as-stationary:
      psum_sT[:, sb*32:+32] += x_half[:, sb*128:+128].T @ q_effT_half
    16 s-blocks packed per [128, 512] PSUM bank; the output IS pT-oriented,
    so p never needs transposing.
  - p = Exp(scale*scoresT): one [128,512] ScalarE activation per chunk -> fp16.
  - l partials: ones[128,1].T @ pT_chunk -> psum_l [1, 512], accumulated
    across chunks (host sums the 16 slots).
  - y: per s-block and c-half: xt_slice[s,128].T @ pT_slice[s,32]
    -> psum_y [128(c-half), 64(2*hq)], PSUM-accumulated over all s.
  - host reassembles y/l and runs the tiny epilogue.
"""

import sys

if "/opt/trn_rl_repo" not in sys.path:
    sys.path.insert(0, "/opt/trn_rl_repo")

import numpy as np

NUM_HEADS = 8
OUT_FEATURES = 512
NUM_QUERIES = 4
C = 256
HEAD_DIM = OUT_FEATURES // NUM_HEADS
LN_EPS = 1e-5
B = 4
S = 32 * 32 * 32
N_CORES = 8
S_LOC = S // 2  # shard: (batch, half of spatial axis)
HQ = NUM_HEADS * NUM_QUERIES  # 32 fused query rows, hq = h*NUM_QUERIES + q
SCALE = HEAD_DIM ** -0.5
CHUNK = 2048

_NC_CACHE = {}


def _build_nc(s_loc=S_LOC, chunk=CHUNK, loop_n=1, x_f32=False, multi_queue=True, dbg_swap_h=False, dbg_no_l=False):
    import concourse.bass as bass
    import concourse.tile as tile
    from concourse import bacc, mybir
    import contextlib

    f32 = mybir.dt.float32
    f16 = mybir.dt.float16
    xdt = f32 if x_f32 else f16
    Exp = mybir.ActivationFunctionType.Exp

    assert s_loc % chunk == 0 and chunk % 512 == 0
    n_ch = s_loc // chunk
    sbpc = chunk // 128     # s-blocks per chunk (16 for chunk=2048)
    n_sb = s_loc // 128
    W = 256

    nc = bacc.Bacc("TRN2", target_bir_lowering=False, debug=False,
                   num_devices=N_CORES)
    x_d = nc.dram_tensor("x", [C, s_loc], xdt, kind="ExternalInput")
    xt_d = nc.dram_tensor("xt", [128, n_sb * W], f16, kind="ExternalInput")
    qT_d = nc.dram_tensor("qT", [C, HQ], xdt, kind="ExternalInput")
    y_d = nc.dram_tensor("y", [128, 2 * HQ], f32, kind="ExternalOutput")
    l_d = nc.dram_tensor("l", [1, 512], f32, kind="ExternalOutput")

    with tile.TileContext(nc) as tc:
        with (
            tc.tile_pool(name="const", bufs=1) as constp,
            tc.tile_pool(name="xstage", bufs=3) as xstage,
            tc.tile_pool(name="xtstage", bufs=3) as xtstage,
            tc.tile_pool(name="ptstage", bufs=3) as ptstage,
            tc.tile_pool(name="outp", bufs=2) as outp,
            tc.tile_pool(name="ps_st", bufs=3, space="PSUM") as ps_st,
            tc.tile_pool(name="ps_y", bufs=1, space="PSUM") as ps_yp,
            tc.tile_pool(name="ps_l", bufs=1, space="PSUM") as ps_lp,
        ):
            qt0 = constp.tile([128, HQ], xdt)
            nc.sync.dma_start(qt0[:], qT_d[0:128, :])
            qt1 = constp.tile([128, HQ], xdt)
            nc.sync.dma_start(qt1[:], qT_d[128:256, :])
            ones = constp.tile([128, 1], f16)
            nc.gpsimd.memset(ones[:], 1.0)

            def iter_scope():
                if loop_n > 1:
                    E = mybir.EngineType
                    return tc.For_i(0, loop_n, 1,
                                    hint_engines=(E.PE, E.DVE, E.Activation,
                                                  E.SP, E.Pool))
                return contextlib.nullcontext()

            q0 = nc.sync
            q1 = nc.scalar if multi_queue else nc.sync
            with iter_scope():
                psum_y = ps_yp.tile([128, 2 * HQ], f32, tag="psy")
                psum_l = ps_lp.tile([1, 512], f32, tag="psl")

                for k in range(n_ch):
                    o = k * chunk
                    xc0 = xstage.tile([128, chunk], xdt, tag="xc0")
                    q0.dma_start(xc0[:], x_d[0:128, o:o + chunk])
                    xc1 = xstage.tile([128, chunk], xdt, tag="xc1")
                    q0.dma_start(xc1[:], x_d[128:256, o:o + chunk])
                    xt_c = xtstage.tile([128, sbpc * W], f16, tag="xt")
                    ocol = (o // 128) * W
                    q1.dma_start(xt_c[:], xt_d[:, ocol:ocol + sbpc * W])

                    # scoresT: 16 s-blocks packed into one [128, 512] bank
                    pst = ps_st.tile([128, sbpc * HQ], f32, tag="pst")
                    for sb in range(sbpc):
                        for h, (xc, qt) in enumerate(((xc0, qt0), (xc1, qt1))):
                            nc.tensor.matmul(
                                pst[:, sb * HQ:(sb + 1) * HQ],
                                xc[:, sb * 128:(sb + 1) * 128], qt[:],
                                start=(sb == 0 and h == 0),
                                stop=(sb == sbpc - 1 and h == 1),
                                skip_group_check=True)
                    pt_c = ptstage.tile([128, sbpc * HQ], f16, tag="pt")
                    nc.scalar.activation(pt_c[:], pst[:], Exp, scale=SCALE)
                    # l partials: sum over the 128 s-rows of this chunk
                    if not dbg_no_l:
                        nc.tensor.matmul(psum_l[:, 0:sbpc * HQ], ones[:],
                                         pt_c[:],
                                         start=(k == 0), stop=(k == n_ch - 1),
                                         skip_group_check=True)
                    else:
                        nc.gpsimd.memset(psum_l[:], 0.0) if k == 0 else None
                    # y: [c-half, hq] accumulated over all s-blocks
                    for sb in range(sbpc):
                        gsb = k * sbpc + sb
                        for h in ((1, 0) if dbg_swap_h else (0, 1)):
                            nc.tensor.matmul(
                                psum_y[:, h * HQ:(h + 1) * HQ],
                                xt_c[:, sb * W + h * 128:sb * W + (h + 1) * 128],
                                pt_c[:, sb * HQ:(sb + 1) * HQ],
                                start=(gsb == 0 and h == (1 if dbg_swap_h else 0)),
                                stop=(gsb == n_sb - 1 and h == (0 if dbg_swap_h else 1)),
                                skip_group_check=True)

                y_t = outp.tile([128, 2 * HQ], f32, tag="yt")
                nc.vector.tensor_copy(y_t[:], psum_y[:])
                l_t = outp.tile([1, 512], f32, tag="lt")
                nc.vector.tensor_copy(l_t[:], psum_l[:])
                nc.sync.dma_start(y_d[:], y_t[:])
                nc.sync.dma_start(l_d[:], l_t[:])

    nc.compile()
    return nc


def _get_nc(loop_n=1, x_f32=False, chunk=CHUNK, multi_queue=True):
    key = (S_LOC, loop_n, x_f32, chunk, multi_queue)
    if key not in _NC_CACHE:
        _NC_CACHE[key] = _build_nc(loop_n=loop_n, x_f32=x_f32, chunk=chunk,
                                   multi_queue=multi_queue)
    return _NC_CACHE[key]


def _shard_inputs(shard, qT, s_loc=S_LOC, chunk=CHUNK, x_f32=False):
    """shard: [C, s_loc] fp32 -> in_map for one core."""
    n_sb = s_loc // 128
    x16 = shard.astype(np.float32 if x_f32 else np.float16)
    # flat transposed plane: column block j (width C) = shard[:, j*128:+128].T
    xt = np.ascontiguousarray(
        shard.T.reshape(n_sb, 128, C).astype(np.float16)
        .transpose(1, 0, 2).reshape(128, n_sb * C))
    return {"x": np.ascontiguousarray(x16), "xt": xt,
            "qT": qT.astype(np.float32 if x_f32 else np.float16)}


def _prepare_in_maps(x, queries, Wk, x_f32=False):
    xf = np.ascontiguousarray(np.asarray(x, np.float32).reshape(B, C, S))
    qr = np.asarray(queries, np.float32).reshape(NUM_QUERIES, NUM_HEADS, HEAD_DIM)
    Wkr = np.asarray(Wk, np.float32).reshape(NUM_HEADS, HEAD_DIM, C)
    # q_eff[h*NQ+q, c] = sum_d q[q,h,d] * Wk[h*hd+d, c]
    q_eff = np.einsum("qhd,hdc->hqc", qr, Wkr).reshape(HQ, C)
    qT = np.ascontiguousarray(q_eff.T.astype(np.float32))
    in_maps = []
    for core in range(N_CORES):
        b, half = divmod(core, 2)
        shard = np.ascontiguousarray(xf[b, :, half * S_LOC:(half + 1) * S_LOC])
        in_maps.append(_shard_inputs(shard, qT, x_f32=x_f32))
    return in_maps


def _extract_yl(yv, lv):
    """Device outputs -> (Y [HQ, C], L [HQ]) for one core."""
    Y = np.concatenate([yv[:, 0:HQ].T, yv[:, HQ:2 * HQ].T], axis=1)  # [HQ, 256]
    L = lv.reshape(-1, HQ).sum(axis=0)
    return Y, L


def _epilogue(Y, L, Wv, bv, Wo, bo, gamma, beta):
    """Y [B, HQ, C], L [B, HQ] -> final [B, OUT_FEATURES]."""
    pooled = (Y / L[:, :, None]).reshape(B, NUM_HEADS, NUM_QUERIES, C)
    Wvr = np.asarray(Wv, np.float32).reshape(NUM_HEADS, HEAD_DIM, C)
    att = np.einsum("hdc,bhqc->bhqd", Wvr, pooled)
    att += np.asarray(bv, np.float32).reshape(1, NUM_HEADS, 1, HEAD_DIM)
    multi = att.transpose(0, 2, 1, 3).reshape(B, NUM_QUERIES * OUT_FEATURES)
    out = multi @ np.asarray(Wo, np.float32).T + np.asarray(bo, np.float32)
    mu = out.mean(-1, keepdims=True)
    var = ((out - mu) ** 2).mean(-1, keepdims=True)
    out = (out - mu) / np.sqrt(var + LN_EPS)
    out = out * np.asarray(gamma, np.float32) + np.asarray(beta, np.float32)
    return out.astype(np.float32)


def kernel(x, queries, Wk, bk, Wv, bv, Wo, bo, gamma, beta):
    from concourse.bass_utils import run_bass_kernel_spmd

    in_maps = _prepare_in_maps(x, queries, Wk)
    nc = _get_nc()
    res = run_bass_kernel_spmd(nc, in_maps, list(range(N_CORES))).results
    Y = np.zeros((B, HQ, C), np.float32)
    L = np.zeros((B, HQ), np.float32)
    for core in range(N_CORES):
        b = core // 2
        Yc, Lc = _extract_yl(res[core]["y"], res[core]["l"])
        Y[b] += Yc
        L[b] += Lc
    return _epilogue(Y, L, Wv, bv, Wo, bo, gamma, beta)
